# revision 25
# baseline (speedup 1.0000x reference)
"""Trainium2 Bass kernel: segment-mean -> gated MLP -> per-node modulation.

Computes, for h_V [N, D] and sorted batch_id [N] (values in [0, S)):
    seg_sum[s] = sum of h_V rows with batch_id == s ; counts[s]
    c_V = seg_sum / max(counts, 1)
    g   = sigmoid(relu(c_V @ W1 + b1) @ W2 + b2)
    out = h_V * g[batch_id]

Distribution: data-parallel over nodes across 8 NeuronCores; counts and all
segment->chain routing are host-known (batch_id is an input), so per-core
variation lives entirely in host-built indicator matrices and the program is
identical on every core.

Layout: transposed, D on SBUF partitions, nodes along the free dimension.
The host places each core's rows into columns of hvT [128, COLS] fp16,
padding every segment run to a multiple of CHAIN columns so each CHAIN-wide
column block ("chain") contains rows of exactly one segment.

  pass 1: per chain, identity-weight matmuls accumulate CHAIN/FOLD slices
          onto one PSUM bank (TensorE pass-through-add), then one DVE
          tensor_reduce collapses [P, FOLD] to the chain's column sum.
          Every 4th chain instead uses ScalarE's fused activation
          accumulator, off the TensorE critical path.  Stats stream reads a
          separate fp8 copy of the data (half the bytes; the segment means
          only shift the final output by ~1e-4 relative).
  comm:   batch_id is globally sorted, so a segment can straddle at most one
          adjacent core boundary.  Boundary-segment chains are laid out
          first; after the first K0 chains two 2-core-group AllReduces
          exchange the straddling partial sums while the rest of pass 1
          still streams.  Interior segments need no communication at all,
          so no global collective sits on the critical path.
  MLP:    on c_V^T [D, S] (the chain->segment matmul, with 1/count folded
          into the host-built selT, produces it directly), all fp32.
  pass 2: per chain, one DVE tensor_scalar multiply (4x mode) by the
          per-partition gate column; fp16 in / fp16 out, host upcasts.
          The last RES_CHAINS chains skip the fp8 stats read entirely:
          their fp16 tiles are loaded once in pass 1, used for stats, kept
          resident in SBUF, and multiplied in place during pass 2.
"""

import math

import numpy as np

# Problem constants (hardcoded per the harness contract).
D = 128  # feature dim
S = 64  # number of segments
P = 128  # SBUF partitions
N_CORES = 8
N_FULL = 1_000_000
ROWS_PER_CORE = N_FULL // N_CORES  # 125000
CHAIN = 2048  # columns per chain (pass-1/pass-2 work unit)
FOLD = 512  # PSUM fold width (one fp32 PSUM bank)
RES_CHAINS = 24  # trailing chains kept resident in SBUF between passes
NB = 8  # boundary-table columns (core boundaries 0..6, padded to 8)


def segment_kernel(tc, outs, ins, n_cores, nch, k0, n_res):
    """Emit the per-core Tile program (uniform across cores)."""
    import concourse.mybir as mybir

    nc = tc.nc
    F32 = mybir.dt.float32
    F16 = mybir.dt.float16
    AF = mybir.ActivationFunctionType
    OP = mybir.AluOpType

    hvT = ins["hvT"]  # [P, nch*CHAIN] fp16, transposed node data
    hvTs = ins["hvTs"]  # [P, nch*CHAIN] fp8 copy for the stats stream
    identS = ins["identS"]  # [P, P] identity, stats dtype
    identF = ins["identF"]  # [P, P] identity, fp16
    selT = ins["selT"]  # [P, S] f32 chain->seg, scaled by 1/count
    selAB = ins["selAB"]  # [P, NB] f32 boundary-table routing
    sel2 = ins["sel2"]  # [S, P] f32 seg->chain gate gather
    w1 = ins["W1"]  # [D, D] f32
    b1 = ins["b1"]  # [D]
    w2 = ins["W2"]
    b2b = ins["b2b"]  # [S, D] f32: b2 broadcast along segments
    ident = ins["ident"]  # [P, P] f32 identity
    outT = outs["out"]  # [P, nch*CHAIN] fp16

    n_fp8 = nch - n_res  # chains streamed from the fp8 copy
    nfold = CHAIN // FOLD

    with tc.tile_pool(name="persist", bufs=1) as pers:
        ident_sb = pers.tile_from(ident, name="ident_sb", force_copy=True)
        identS_sb = pers.tile_from(identS, name="identS_sb", force_copy=True)
        identF_sb = pers.tile_from(identF, name="identF_sb", force_copy=True)
        w1_sb = pers.tile_from(w1, name="w1_sb", force_copy=True)
        w2_sb = pers.tile_from(w2, name="w2_sb", force_copy=True)
        selT_sb = pers.tile_from(selT, name="selT_sb", force_copy=True)
        selAB_sb = pers.tile_from(selAB, name="selAB_sb", force_copy=True)
        sel2_sb = pers.tile_from(sel2, name="sel2_sb", force_copy=True)
        b1_sb = pers.tile([P, 1], F32, name="b1_sb")
        nc.sync.dma_start(out=b1_sb, in_=b1)
        b2b_sb = pers.tile_from(b2b, name="b2b_sb", force_copy=True)
        gsum = pers.tile([P, nch + NB], F32, name="gsum")
        gate_sb = pers.tile([P, nch], F32, name="gate_sb")

        with (
            tc.tile_pool(name="p1hv", bufs=10) as hv1p,
            tc.tile_pool(name="resp", bufs=n_res) as resp,
            tc.tile_pool(name="junkp", bufs=1) as junkp,
            tc.tile_pool(name="foldps", bufs=4, space="PSUM") as foldp,
            tc.tile_pool(name="mlp", bufs=2) as mlp_sb,
            tc.tile_pool(name="mlpps", bufs=2, space="PSUM") as mlp_ps,
            tc.tile_pool(name="ccdram", bufs=1, space="DRAM") as dramp,
            tc.tile_pool(name="p2hv", bufs=10) as hv2p,
            tc.tile_pool(name="p2out", bufs=6) as outp,
        ):
            junk = junkp.tile([P, CHAIN], F16, name="junk")
            res_tiles = {}

            def stats_chain(c):
                """Emit pass-1 stats for chain c."""
                if c < n_fp8:
                    hv_t = hv1p.tile([P, CHAIN], hvTs.tensor.dtype, tag="hv1",
                                     name=f"hv1_{c}")
                    nc.sync.dma_start(
                        out=hv_t, in_=hvTs[:, c * CHAIN : (c + 1) * CHAIN]
                    )
                    id_sb = identS_sb
                else:
                    hv_t = resp.tile([P, CHAIN], F16, tag="res", name=f"res_{c}")
                    nc.sync.dma_start(
                        out=hv_t, in_=hvT[:, c * CHAIN : (c + 1) * CHAIN]
                    )
                    res_tiles[c] = hv_t
                    id_sb = identF_sb
                if c % 4 == 3:
                    nc.scalar.activation(
                        junk, hv_t, AF.Copy, accum_out=gsum[:, c : c + 1]
                    )
                    return
                fold_ps = foldp.tile([P, FOLD], F32, tag="fold", name=f"fold_{c}")
                for k in range(nfold):
                    nc.tensor.matmul(
                        fold_ps,
                        lhsT=id_sb,
                        rhs=hv_t[:, k * FOLD : (k + 1) * FOLD],
                        start=(k == 0),
                        stop=(k == nfold - 1),
                    )
                nc.vector.tensor_reduce(
                    gsum[:, c : c + 1], fold_ps, axis=mybir.AxisListType.X, op=OP.add
                )

            # ---- pass 1, boundary chains first, then the pair exchange ----
            for c in range(k0):
                stats_chain(c)

            # Boundary table [P, NB] = gsum[:, :k0]^T-contracted with selAB:
            # column b holds this core's partial sum for the segment
            # straddling core boundary b (zero if not adjacent / no
            # straddle).  One global AllReduce turns every column into that
            # boundary's total, while the rest of pass 1 still streams.
            # Results land in gsum columns nch..nch+NB-1; host-built selT
            # rows route each core's two relevant columns.
            gsumE_ps = mlp_ps.tile([k0, P], F32, name="gsumE_ps", tag="mlpps")
            nc.tensor.transpose(gsumE_ps, gsum[:, :k0], ident_sb)
            gsumE_sb = mlp_sb.tile([k0, P], F32, name="gsumE_sb")
            nc.scalar.copy(gsumE_sb, gsumE_ps)
            bufs_ps = mlp_ps.tile([P, NB], F32, name="bufs_ps", tag="mlpps")
            nc.tensor.matmul(
                bufs_ps, lhsT=gsumE_sb, rhs=selAB_sb[:k0, :], start=True, stop=True
            )
            bufs_sb = mlp_sb.tile([P, NB], F32, name="bufs_sb")
            nc.scalar.copy(bufs_sb, bufs_ps)
            ar_in = dramp.tile([P, NB], F32, name="ar_in")
            ar_out = dramp.tile([P, NB], F32, name="ar_out", addr_space="Local")
            nc.sync.dma_start(out=ar_in, in_=bufs_sb)
            if n_cores > 1:
                nc.gpsimd.collective_compute(
                    "AllReduce",
                    OP.add,
                    replica_groups=[list(range(n_cores))],
                    ins=[ar_in.opt()],
                    outs=[ar_out.opt()],
                )
                nc.sync.dma_start(out=gsum[:, nch : nch + NB], in_=ar_out)
            else:
                nc.sync.dma_start(out=gsum[:, nch : nch + NB], in_=ar_in)

            # ---------------- pass 1, remaining chains ----------------
            for c in range(k0, nch):
                stats_chain(c)

            # ----- chains -> c_V^T [D, S] (selT carries 1/count) -----
            gsumT_ps = mlp_ps.tile([nch + NB, P], F32, name="gsumT_ps", tag="mlpps")
            nc.tensor.transpose(gsumT_ps, gsum, ident_sb)
            gsumT_sb = mlp_sb.tile([nch + NB, P], F32, name="gsumT_sb")
            nc.scalar.copy(gsumT_sb, gsumT_ps)
            cvt_ps = mlp_ps.tile([D, S], F32, name="cvt_ps", tag="mlpps")
            nc.tensor.matmul(
                cvt_ps,
                lhsT=gsumT_sb,
                rhs=selT_sb[: nch + NB, :],
                start=True,
                stop=True,
            )
            cvt_sb = mlp_sb.tile([D, S], F32, name="cvt_sb")
            nc.scalar.copy(cvt_sb, cvt_ps)

            # ---------------- replicated MLP on c_V^T [D, S] ----------------
            h1_ps = mlp_ps.tile([D, S], F32, name="h1_ps", tag="mlpps")
            nc.tensor.matmul(h1_ps, lhsT=w1_sb, rhs=cvt_sb, start=True, stop=True)
            h1_sb = mlp_sb.tile([D, S], F32, name="h1_sb")
            nc.scalar.activation(h1_sb, h1_ps, AF.Relu, bias=b1_sb, scale=1.0)
            # h2[s, k] = sum_j h1T[j, s] W2[j, k]; + b2 broadcast; sigmoid.
            h2_ps = mlp_ps.tile([S, D], F32, name="h2_ps", tag="mlpps")
            nc.tensor.matmul(h2_ps, lhsT=h1_sb, rhs=w2_sb, start=True, stop=True)
            h2_sb = mlp_sb.tile([S, D], F32, name="h2_sb")
            nc.vector.tensor_tensor(h2_sb, h2_ps, b2b_sb, OP.add)
            g_sb = mlp_sb.tile([S, D], F32, name="g_sb")
            nc.scalar.activation(g_sb, h2_sb, AF.Sigmoid, bias=0.0, scale=1.0)
            gate_ps = mlp_ps.tile([P, nch], F32, name="gate_ps", tag="mlpps")
            nc.tensor.matmul(
                gate_ps, lhsT=g_sb, rhs=sel2_sb[:, :nch], start=True, stop=True
            )
            nc.scalar.copy(gate_sb, gate_ps)

            # ---------------- pass 2: gate and store ----------------
            for c in range(nch):
                if c in res_tiles:
                    hv_t = res_tiles[c]
                else:
                    hv_t = hv2p.tile([P, CHAIN], F16, tag="hv2", name=f"hv2_{c}")
                    nc.sync.dma_start(
                        out=hv_t, in_=hvT[:, c * CHAIN : (c + 1) * CHAIN]
                    )
                out_t = outp.tile([P, CHAIN], F16, tag="out", name=f"out_{c}")
                nc.vector.tensor_scalar(
                    out_t, hv_t, gate_sb[:, c : c + 1], None, OP.mult
                )
                nc.sync.dma_start(
                    out=outT[:, c * CHAIN : (c + 1) * CHAIN], in_=out_t
                )


def build_nc(n_cores, nch, k0, n_res):
    """Build the full Bass module with ExternalInput/Output DRAM tensors."""
    import concourse.bacc as bacc
    import concourse.mybir as mybir
    import concourse.tile as tile

    F32 = mybir.dt.float32
    F16 = mybir.dt.float16
    F8 = mybir.dt.float8e4
    cols = nch * CHAIN
    nc = bacc.Bacc(
        "TRN2",
        target_bir_lowering=False,
        debug=False,
        enable_asserts=False,
        num_devices=n_cores,
    )

    def din(name, shape, dt):
        return nc.dram_tensor(name, shape, dt, kind="ExternalInput").ap()

    ins = {
        "hvT": din("hvT", [P, cols], F16),
        "hvTs": din("hvTs", [P, cols], F8),
        "identS": din("identS", [P, P], F8),
        "identF": din("identF", [P, P], F16),
        "selT": din("selT", [P, S], F32),
        "selAB": din("selAB", [P, NB], F32),
        "sel2": din("sel2", [S, P], F32),
        "W1": din("W1", [D, D], F32),
        "b1": din("b1", [D], F32),
        "W2": din("W2", [D, D], F32),
        "b2b": din("b2b", [S, D], F32),
        "ident": din("ident", [P, P], F32),
    }
    outs = {"out": nc.dram_tensor("out", [P, cols], F16, kind="ExternalOutput").ap()}
    with tile.TileContext(nc) as tc:
        segment_kernel(tc, outs, ins, n_cores, nch, k0, n_res)
    nc.compile()
    return nc


def _core_layout(bid_core):
    """Runs (seg, start, len) of one core's sorted bid shard, boundary-first.

    Returns (ordered_runs, nch, n_boundary_chains).  ordered_runs puts the
    first-row segment's run and the last-row segment's run first so the
    cross-core exchange can fire early.
    """
    segs, starts = np.unique(bid_core, return_index=True)
    starts = list(starts) + [len(bid_core)]
    runs = []
    for i, s in enumerate(segs):
        runs.append((int(s), int(starts[i]), int(starts[i + 1] - starts[i])))
    assert len(runs) >= 3, "expected several segments per core"
    ordered = [runs[0], runs[-1]] + runs[1:-1]
    nch = sum(math.ceil(ln / CHAIN) for _, _, ln in ordered)
    kb = math.ceil(runs[0][2] / CHAIN) + math.ceil(runs[-1][2] / CHAIN)
    return ordered, nch, kb


_NC_CACHE = {}


def _get_nc(nch, k0, n_res):
    key = (N_CORES, nch, k0, n_res)
    if key not in _NC_CACHE:
        _NC_CACHE[key] = build_nc(*key)
    return _NC_CACHE[key]


def run(inputs, trace=False, trace_kwargs=None):
    import ml_dtypes

    from concourse import bass_utils

    h_V = np.asarray(inputs["h_V"], dtype=np.float32)
    bid = np.asarray(inputs["batch_id"]).astype(np.int64)
    counts = np.bincount(bid, minlength=S).astype(np.float64)
    inv_cnt = (1.0 / np.maximum(counts, 1.0)).astype(np.float32)
    b2 = np.asarray(inputs["b2"], np.float32)
    weights = {
        "W1": np.ascontiguousarray(np.asarray(inputs["W1"], np.float32)),
        "b1": np.ascontiguousarray(np.asarray(inputs["b1"], np.float32)),
        "W2": np.ascontiguousarray(np.asarray(inputs["W2"], np.float32)),
        "b2b": np.ascontiguousarray(np.broadcast_to(b2, (S, D))),
        "ident": np.eye(P, dtype=np.float32),
        "identS": np.eye(P, dtype=ml_dtypes.float8_e4m3),
        "identF": np.eye(P, dtype=np.float16),
    }

    hvT_all = np.ascontiguousarray(h_V.astype(np.float16).T)  # [128, N]
    hvTs_all = hvT_all.astype(ml_dtypes.float8_e4m3)

    # Which core boundaries are straddled by a segment (globally sorted bid).
    straddle = [
        bid[(b + 1) * ROWS_PER_CORE - 1] == bid[(b + 1) * ROWS_PER_CORE]
        for b in range(N_CORES - 1)
    ]

    core_layouts = []
    nch = 0
    k0 = 0
    for c in range(N_CORES):
        lo, hi = c * ROWS_PER_CORE, (c + 1) * ROWS_PER_CORE
        runs, nch_c, kb_c = _core_layout(bid[lo:hi])
        core_layouts.append(runs)
        nch = max(nch, nch_c)
        k0 = max(k0, kb_c)
    assert nch + NB <= P, f"chain count {nch}+{NB} exceeds {P}"
    n_res = min(RES_CHAINS, nch - k0 - 4)
    cols = nch * CHAIN
    n_fp8 = nch - n_res

    in_maps = []
    core_cols = []  # per core: list of (r0, ln, col) for unmarshal
    for c in range(N_CORES):
        lo = c * ROWS_PER_CORE
        runs = core_layouts[c]
        # Boundary-table routing: a segment straddling core boundary b
        # (between cores b and b+1) is summed via table column b instead of
        # the local selT.  Core c's left boundary is c-1, right is c.
        # Run order is [L-run, R-run, interiors...].
        exch = {}  # table column -> seg
        if c > 0 and straddle[c - 1]:
            exch[c - 1] = runs[0][0]
        if c < N_CORES - 1 and straddle[c]:
            exch[c] = runs[1][0]
        exch_segs = set(exch.values())

        hvT = np.zeros((P, cols), np.float16)
        hvTs = np.zeros((P, cols), ml_dtypes.float8_e4m3)
        selT = np.zeros((P, S), np.float32)
        selAB = np.zeros((P, NB), np.float32)
        sel2 = np.zeros((S, P), np.float32)
        col = 0
        ch = 0
        rcols = []
        for s, r0, ln in runs:
            hvT[:, col : col + ln] = hvT_all[:, lo + r0 : lo + r0 + ln]
            n_ch = math.ceil(ln / CHAIN)
            if ch < n_fp8:  # stats stream needs the fp8 copy
                e = min(col + n_ch * CHAIN, n_fp8 * CHAIN)
                ln_s = min(ln, max(0, e - col))
                hvTs[:, col : col + ln_s] = hvTs_all[:, lo + r0 : lo + r0 + ln_s]
            if s in exch_segs:
                # routed through the boundary table, not the local selT
                for b, seg in exch.items():
                    if seg == s:
                        selAB[ch : ch + n_ch, b] = 1.0
            else:
                selT[ch : ch + n_ch, s] = inv_cnt[s]
            sel2[s, ch : ch + n_ch] = 1.0
            rcols.append((r0, ln, col))
            col += n_ch * CHAIN
            ch += n_ch
        # boundary-table totals occupy gsum columns nch..nch+NB-1
        for b, seg in exch.items():
            selT[nch + b, seg] = inv_cnt[seg]
        core_cols.append(rcols)
        in_maps.append(
            {
                "hvT": hvT,
                "hvTs": hvTs,
                "selT": selT,
                "selAB": selAB,
                "sel2": sel2,
                **weights,
            }
        )

    nc = _get_nc(nch, k0, n_res)
    res = bass_utils.run_bass_kernel_spmd(
        nc,
        in_maps,
        core_ids=list(range(N_CORES)),
        trace=trace,
        **(trace_kwargs or {}),
    )

    out = np.empty((N_FULL, D), np.float32)
    for c in range(N_CORES):
        lo = c * ROWS_PER_CORE
        outT = res.results[c]["out"]  # [128, cols] fp16
        for r0, ln, col in core_cols[c]:
            out[lo + r0 : lo + r0 + ln] = outT[:, col : col + ln].T
    return out, res


def kernel(**inputs) -> np.ndarray:
    out, _ = run(inputs, trace=False)
    return out
